# revision 1
# baseline (speedup 1.0000x reference)
"""nn_Cvx_ShortestPathNet — TRN2 Bass kernel, 8-core pure data parallelism.

Math (derived from the reference's Dykstra iteration):
    G = A' pinv(AA') A   (orthogonal projector, 760x760), c = b' pinv(AA') A
    w = MLP(d); t_1 = -w
    for k = 1..K:  corr_k = t_k @ G - c ;  t_{k+1} = max(-w, corr_k)
    y = max(-w - corr_K, 0)
(The invariant t2 + p == -w collapses Dykstra's three-sequence state to a
single iterate.)

On-chip layout is transposed ([n2, B_local], n2 padded 760->768 = 6x128
partition tiles) so the per-iteration matmul runs with M=128 on the PE:
    corr^T[j] = sum_k (G block k,j as lhsT) @ t^T[k]
Per iteration: 36 matmuls (PE) + 6 fused scalar_tensor_tensor ops (DVE):
    t_next[j] = max(psum[j] - c[j], negw[j])

Batch 256 is sharded 32 rows per core; A, G, c and MLP weights replicated.
"""

import json
import numpy as np

import concourse.bass as bass
import concourse.mybir as mybir
import concourse.tile as tile
from concourse.bass_utils import run_bass_kernel_spmd

F32 = mybir.dt.float32
AT = mybir.AluOpType
AF = mybir.ActivationFunctionType

JT = 6          # 768/128 edge-dim tiles
BL = 32         # batch rows per core
HT = 5          # 640/128 hidden tiles
K_ITERS = 100
N_CORES = 8
N2 = 760
MM_DTYPE = mybir.dt.float16   # G and t in fp16: 7.4x faster than fp32 on PE
MM_NP = np.float16

# ---------------------------------------------------------------------------
# This container's walrus build rejects instructions carrying more than one
# sync-wait. Split any multi-wait instruction at the BIR-JSON level: insert
# same-engine NoOps before it, each carrying one of the extra waits (waits
# are sem-ge, so order is irrelevant).
_orig_to_json_bytes = bass.Bass.to_json_bytes
_ctr = [0]


def _split_waits_json(raw: bytes) -> bytes:
    j = json.loads(raw)
    changed = False
    for fn in j.get("functions", []):
        for bb in fn.get("blocks", []):
            out = []
            for inst in bb.get("instructions", []):
                si = inst.get("sync_info") or {}
                waits = si.get("on_wait") or []
                if len(waits) > 1:
                    changed = True
                    for w in waits[:-1]:
                        _ctr[0] += 1
                        out.append({
                            "debug": inst.get("debug", 0),
                            "engine": inst["engine"],
                            "ins": [], "outs": [],
                            "name": f"I-waitsplit-{_ctr[0]}",
                            "opcode": "NoOp",
                            "sync_info": {"on_wait": [w], "on_update": []},
                        })
                    si["on_wait"] = waits[-1:]
                out.append(inst)
            bb["instructions"] = out
    return json.dumps(j).encode() if changed else raw


def _patched_to_json_bytes(self, *a, **k):
    return _split_waits_json(_orig_to_json_bytes(self, *a, **k))


bass.Bass.to_json_bytes = _patched_to_json_bytes


def _build(mm_dtype=F32, k_iters=K_ITERS):
    nc = bass.Bass("TRN2", target_bir_lowering=False, debug=False,
                   num_devices=N_CORES)
    DT = mm_dtype

    g_mat = nc.dram_tensor("g_mat", [128, JT * JT * 128], DT, kind="ExternalInput").ap()
    w2t = nc.dram_tensor("w2t", [128, HT * JT * 128], F32, kind="ExternalInput").ap()
    w1 = nc.dram_tensor("w1", [64, HT * 128], F32, kind="ExternalInput").ap()
    dt_in = nc.dram_tensor("dt_in", [64, BL], F32, kind="ExternalInput").ap()
    ccols = nc.dram_tensor("ccols", [128, JT], F32, kind="ExternalInput").ap()
    b1c = nc.dram_tensor("b1c", [128, HT], F32, kind="ExternalInput").ap()
    nb2c = nc.dram_tensor("nb2c", [128, JT], F32, kind="ExternalInput").ap()
    y_out = nc.dram_tensor("y_out", [128, JT * BL], F32, kind="ExternalOutput").ap()

    with tile.TileContext(nc) as tc:
        with (
            tc.tile_pool(name="const", bufs=1) as cpool,
            tc.tile_pool(name="state", bufs=2) as spool,
            tc.tile_pool(name="psum", bufs=2, space="PSUM") as ppool,
        ):
            dT_sb = cpool.tile([64, BL], F32)
            nc.sync.dma_start(out=dT_sb[:], in_=dt_in[:])
            w1_sb = cpool.tile([64, HT * 128], F32)
            nc.sync.dma_start(out=w1_sb[:], in_=w1[:])
            ccols_sb = cpool.tile([128, JT], F32)
            nc.sync.dma_start(out=ccols_sb[:], in_=ccols[:])
            b1c_sb = cpool.tile([128, HT], F32)
            nc.sync.dma_start(out=b1c_sb[:], in_=b1c[:])
            nb2c_sb = cpool.tile([128, JT], F32)
            nc.sync.dma_start(out=nb2c_sb[:], in_=nb2c[:])
            w2_sb = cpool.tile([128, HT * JT * 128], F32)
            nc.sync.dma_start(out=w2_sb[:], in_=w2t[:])
            # G on the SWDGE path so it overlaps the W2 load
            G_sb = cpool.tile([128, JT * JT * 128], DT)
            nc.gpsimd.dma_start(out=G_sb[:], in_=g_mat[:])

            # MLP: h = leaky_relu(d@W1 + b1), negw = -(h@W2 + b2)
            h_sb = cpool.tile([128, HT * BL], F32)
            for m in range(HT):
                ph = ppool.tile([128, BL], F32, tag="mlp")
                nc.tensor.matmul(out=ph[:], lhsT=w1_sb[:, m * 128:(m + 1) * 128],
                                 rhs=dT_sb[:], start=True, stop=True)
                pre = spool.tile([128, BL], F32, tag="pre", name=f"pre{m}")
                nc.scalar.activation(out=pre[:], in_=ph[:], func=AF.Identity,
                                     bias=b1c_sb[:, m:m + 1], scale=1.0)
                # leaky relu: Lrelu's alpha operand is ignored by this
                # compiler build (hardcodes 0.01), so do max(x, 0.1x) on DVE
                nc.vector.scalar_tensor_tensor(
                    out=h_sb[:, m * BL:(m + 1) * BL], in0=pre[:],
                    scalar=0.1, in1=pre[:], op0=AT.mult, op1=AT.max)
            negw = cpool.tile([128, JT * BL], F32)
            for j in range(JT):
                pw = ppool.tile([128, BL], F32, tag="mlp")
                for k2 in range(HT):
                    nc.tensor.matmul(
                        out=pw[:],
                        lhsT=w2_sb[:, (k2 * JT + j) * 128:(k2 * JT + j + 1) * 128],
                        rhs=h_sb[:, k2 * BL:(k2 + 1) * BL],
                        start=(k2 == 0), stop=(k2 == HT - 1))
                nc.scalar.activation(out=negw[:, j * BL:(j + 1) * BL], in_=pw[:],
                                     func=AF.Identity, bias=nb2c_sb[:, j:j + 1],
                                     scale=-1.0)

            t_cur = [spool.tile([128, BL], DT, tag=f"t{k}", name=f"tcur{k}")
                     for k in range(JT)]
            for k in range(JT):
                nc.vector.tensor_copy(out=t_cur[k][:],
                                      in_=negw[:, k * BL:(k + 1) * BL])

            for it in range(k_iters):
                ps = [ppool.tile([128, 2 * BL], F32, tag=f"ps{p}",
                                 name=f"ps{it}_{p}") for p in range(3)]
                for j in range(JT):
                    pj = ps[j // 2]
                    sl = slice((j % 2) * BL, (j % 2 + 1) * BL)
                    for k in range(JT):
                        nc.tensor.matmul(
                            out=pj[:, sl],
                            lhsT=G_sb[:, (k * JT + j) * 128:(k * JT + j + 1) * 128],
                            rhs=t_cur[k][:],
                            start=(k == 0), stop=(k == JT - 1))
                if it < k_iters - 1:
                    t_nxt = [spool.tile([128, BL], DT, tag=f"t{k}",
                                        name=f"t{it}_{k}") for k in range(JT)]
                    for j in range(JT):
                        nc.vector.scalar_tensor_tensor(
                            out=t_nxt[j][:],
                            in0=ps[j // 2][:, (j % 2) * BL:(j % 2 + 1) * BL],
                            scalar=ccols_sb[:, j:j + 1],
                            in1=negw[:, j * BL:(j + 1) * BL],
                            op0=AT.subtract, op1=AT.max)
                    t_cur = t_nxt
                else:
                    y_sb = cpool.tile([128, JT * BL], F32)
                    for j in range(JT):
                        z = spool.tile([128, BL], F32, tag="z", name=f"z{j}")
                        nc.vector.scalar_tensor_tensor(
                            out=z[:],
                            in0=ps[j // 2][:, (j % 2) * BL:(j % 2 + 1) * BL],
                            scalar=ccols_sb[:, j:j + 1],
                            in1=negw[:, j * BL:(j + 1) * BL],
                            op0=AT.subtract, op1=AT.subtract)
                        nc.scalar.activation(out=y_sb[:, j * BL:(j + 1) * BL],
                                             in_=z[:], func=AF.Relu, scale=-1.0)
                    nc.sync.dma_start(out=y_out[:], in_=y_sb[:])
    return nc


def _host_prepare(d, W1, b1, W2, b2, A, b_eq, mm_np_dtype=np.float32):
    A64 = A.astype(np.float64)
    M = np.linalg.pinv(A64 @ A64.T)
    G = A64.T @ M @ A64
    c = (b_eq.astype(np.float64) @ M) @ A64

    n2 = A.shape[1]
    NP = JT * 128
    G_pad = np.zeros((NP, NP), np.float64)
    G_pad[:n2, :n2] = G
    c_pad = np.zeros(NP, np.float64)
    c_pad[:n2] = c

    g_sb = (G_pad.reshape(JT, 128, JT, 128).transpose(1, 0, 2, 3)
            .reshape(128, JT * JT * 128)).astype(mm_np_dtype)
    ccols = c_pad.reshape(JT, 128).T.astype(np.float32).copy()

    HID = W1.shape[1]
    W2_pad = np.zeros((HID, NP), np.float64)
    W2_pad[:, :n2] = W2.astype(np.float64)
    w2_sb = (W2_pad.reshape(HT, 128, JT, 128).transpose(1, 0, 2, 3)
             .reshape(128, HT * JT * 128)).astype(np.float32)
    b1c = b1.reshape(HT, 128).T.astype(np.float32).copy()
    b2_pad = np.zeros(NP, np.float32)
    b2_pad[:n2] = b2
    nb2c = (-b2_pad).reshape(JT, 128).T.astype(np.float32).copy()

    shared = {"g_mat": g_sb, "w2t": w2_sb, "w1": W1.astype(np.float32),
              "ccols": ccols, "b1c": b1c, "nb2c": nb2c}
    B = d.shape[0]
    bl = B // N_CORES
    in_maps = []
    for i in range(N_CORES):
        dT = d[i * bl:(i + 1) * bl, :].T.astype(np.float32).copy()
        in_maps.append({**shared, "dt_in": dT})
    return in_maps


_nc_cache = {}


def kernel(d, W1, b1, W2, b2, A, b_eq):
    d = np.asarray(d, np.float32)
    W1 = np.asarray(W1, np.float32)
    b1 = np.asarray(b1, np.float32)
    W2 = np.asarray(W2, np.float32)
    b2 = np.asarray(b2, np.float32)
    A = np.asarray(A, np.float32)
    b_eq = np.asarray(b_eq, np.float32)

    if "nc" not in _nc_cache:
        _nc_cache["nc"] = _build(mm_dtype=MM_DTYPE)
    nc = _nc_cache["nc"]

    in_maps = _host_prepare(d, W1, b1, W2, b2, A, b_eq, mm_np_dtype=MM_NP)
    res = run_bass_kernel_spmd(nc, in_maps, list(range(N_CORES)))

    outs = []
    for r in res.results:
        y = (r["y_out"].reshape(128, JT, BL).transpose(2, 1, 0)
             .reshape(BL, JT * 128))
        outs.append(y[:, :N2])
    return np.concatenate(outs, axis=0).astype(np.float32)



# revision 5
# speedup vs baseline: 1.2110x; 1.2110x over previous
"""nn_Cvx_ShortestPathNet — TRN2 Bass kernel, 8-core pure data parallelism.

Math (collapsed Dykstra, c folded into G's padding row):
    G = A' pinv(AA') A   (orthogonal projector, 760x760), c = b' pinv(AA') A
    Key identity: c @ G = c  (since M (AA') M = M), so the affine bias -c can
    be carried by a constant-1 row in t:  Gp[CROW, :n2] = -c, Gp[CROW,CROW]=1,
    t[CROW] == 1  =>  ps = t @ Gp  computes  t@G - c  directly.
    Recurrence:  t_{k+1} = max(negw, ps_k);  y = max(negw - ps_K, 0).

On-chip layout is transposed ([768, B_local], 6x128 partition tiles) so each
matmul runs with M=128, N=32 on the PE. Per iteration: 36 matmuls in three
12-matmul j-pair groups (k-major within a group), each group followed by one
fused DVE max over [128, 64]. The group DVEs overlap the next group's
matmuls, so the PE runs near its ~27ns/matmul issue floor.

PSUM discipline: start=True clears the whole bank's has_written bits, so
only the first matmul into each psum tile carries it; later k=0 matmuls
rely on cleared has_written to overwrite.

MLP: b1 is folded into the W1 matmul via a 65th all-ones contraction row;
W2 is pre-negated on the host and b2 applied as a broadcast tensor on DVE,
so the scalar engine (and its 1.3us ACT table load) is never touched.

Batch 256 is sharded 32 rows per core; G and MLP weights replicated, all in
fp16 (fp32 accumulate in PSUM).
"""

import json
import numpy as np

import concourse.bass as bass
import concourse.mybir as mybir
import concourse.tile as tile
from concourse.bass_utils import run_bass_kernel_spmd

F32 = mybir.dt.float32
AT = mybir.AluOpType
AF = mybir.ActivationFunctionType

JT = 6          # 768/128 edge-dim tiles
BL = 32         # batch rows per core
HT = 5          # 640/128 hidden tiles
K_ITERS = 100
N_CORES = 8
N2 = 760
CROW = 760      # padding row carrying the -c bias (t[CROW] == 1)
MM_DTYPE = mybir.dt.float16
MM_NP = np.float16

# ---------------------------------------------------------------------------
# This container's walrus build rejects instructions carrying more than one
# sync-wait. Split any multi-wait instruction at the BIR-JSON level: insert
# same-engine NoOps before it, each carrying one of the extra waits (waits
# are sem-ge, so order is irrelevant).
_orig_to_json_bytes = bass.Bass.to_json_bytes
_ctr = [0]


def _split_waits_json(raw: bytes) -> bytes:
    j = json.loads(raw)
    changed = False
    for fn in j.get("functions", []):
        for bb in fn.get("blocks", []):
            out = []
            for inst in bb.get("instructions", []):
                si = inst.get("sync_info") or {}
                waits = si.get("on_wait") or []
                if len(waits) > 1:
                    changed = True
                    for w in waits[:-1]:
                        _ctr[0] += 1
                        out.append({
                            "debug": inst.get("debug", 0),
                            "engine": inst["engine"],
                            "ins": [], "outs": [],
                            "name": f"I-waitsplit-{_ctr[0]}",
                            "opcode": "NoOp",
                            "sync_info": {"on_wait": [w], "on_update": []},
                        })
                    si["on_wait"] = waits[-1:]
                out.append(inst)
            bb["instructions"] = out
    return json.dumps(j).encode() if changed else raw


def _patched_to_json_bytes(self, *a, **k):
    return _split_waits_json(_orig_to_json_bytes(self, *a, **k))


bass.Bass.to_json_bytes = _patched_to_json_bytes


def _gpos(j, k):
    """Column block index of G block (k, j) in pair-major g_mat layout."""
    return (j // 2) * 12 + k * 2 + (j % 2)


def _build(mm_dtype=MM_DTYPE, k_iters=K_ITERS):
    nc = bass.Bass("TRN2", target_bir_lowering=False, debug=False,
                   num_devices=N_CORES)
    DT = mm_dtype
    GC = JT * JT * 128          # g_mat columns, pair-major
    CH = 12 * 128               # one pair's worth of G blocks

    dt_in = nc.dram_tensor("dt_in", [65, BL], DT, kind="ExternalInput").ap()
    w1 = nc.dram_tensor("w1", [65, HT * 128], DT, kind="ExternalInput").ap()
    nb2cb = nc.dram_tensor("nb2cb", [128, JT * BL], F32,
                           kind="ExternalInput").ap()
    w2t = nc.dram_tensor("w2t", [128, HT * JT * 128], DT, kind="ExternalInput").ap()
    g_mat = nc.dram_tensor("g_mat", [128, GC], DT, kind="ExternalInput").ap()
    y_out = nc.dram_tensor("y_out", [128, JT * BL], F32, kind="ExternalOutput").ap()

    with tile.TileContext(nc) as tc:
        with (
            tc.tile_pool(name="const", bufs=1) as cpool,
            tc.tile_pool(name="state", bufs=2) as spool,
            tc.tile_pool(name="psum", bufs=2, space="PSUM") as ppool,
        ):
            dT_sb = cpool.tile([65, BL], DT)
            nc.sync.dma_start(out=dT_sb[:], in_=dt_in[:])
            w1_sb = cpool.tile([65, HT * 128], DT)
            nc.sync.dma_start(out=w1_sb[:], in_=w1[:])
            nb2cb_sb = cpool.tile([128, JT * BL], F32)
            nc.sync.dma_start(out=nb2cb_sb[:], in_=nb2cb[:])
            w2_sb = cpool.tile([128, HT * JT * 128], DT)
            nc.sync.dma_start(out=w2_sb[:], in_=w2t[:])
            G_sb = cpool.tile([128, GC], DT)
            for p in range(3):
                nc.sync.dma_start(out=G_sb[:, p * CH:(p + 1) * CH],
                                  in_=g_mat[:, p * CH:(p + 1) * CH])

            # MLP: h = leaky_relu(d@W1 + b1)  (b1 via ones-row of dT)
            h_sb = cpool.tile([128, HT * BL], DT)
            for m in range(HT):
                ph = ppool.tile([128, BL], F32, tag="mlp")
                nc.tensor.matmul(out=ph[:], lhsT=w1_sb[:, m * 128:(m + 1) * 128],
                                 rhs=dT_sb[:], start=True, stop=True)
                # leaky relu: DVE reads at most one PSUM operand, so
                # stage 0.1x in SBUF then max(x, 0.1x)
                pre = spool.tile([128, BL], F32, tag="pre", name=f"pre{m}")
                nc.vector.tensor_scalar_mul(pre[:], ph[:], 0.1)
                nc.vector.scalar_tensor_tensor(
                    out=h_sb[:, m * BL:(m + 1) * BL], in0=ph[:],
                    scalar=1.0, in1=pre[:], op0=AT.mult, op1=AT.max)

            # negw = h@(-W2) + (-b2)  into the three pair psum tiles
            negw = cpool.tile([128, JT * BL], F32)
            t_cur = spool.tile([128, JT * BL], DT, tag="t", name="t_init")
            for p in range(3):
                pw = ppool.tile([128, 2 * BL], F32, tag=f"ps{p}",
                                name=f"pw{p}")
                for jj in range(2):
                    j = 2 * p + jj
                    for k2 in range(HT):
                        nc.tensor.matmul(
                            out=pw[:, jj * BL:(jj + 1) * BL],
                            lhsT=w2_sb[:, (k2 * JT + j) * 128:
                                       (k2 * JT + j + 1) * 128],
                            rhs=h_sb[:, k2 * BL:(k2 + 1) * BL],
                            start=(jj == 0 and k2 == 0), stop=(k2 == HT - 1),
                            skip_group_check=True)
                sl = slice(p * 2 * BL, (p + 1) * 2 * BL)
                nc.vector.scalar_tensor_tensor(
                    out=negw[:, sl], in0=pw[:], scalar=1.0,
                    in1=nb2cb_sb[:, sl], op0=AT.mult, op1=AT.add)
                nc.vector.tensor_copy(out=t_cur[:, sl], in_=negw[:, sl])

            def mm(ps_tile, p, jj, k, rhs_t):
                j = 2 * p + jj
                pos = _gpos(j, k)
                nc.tensor.matmul(
                    out=ps_tile[:, jj * BL:(jj + 1) * BL],
                    lhsT=G_sb[:, pos * 128:(pos + 1) * 128],
                    rhs=rhs_t[:, k * BL:(k + 1) * BL],
                    start=(jj == 0 and k == 0), stop=(k == JT - 1),
                    skip_group_check=True)

            for it in range(k_iters):
                last = it == k_iters - 1
                t_nxt = None if last else spool.tile(
                    [128, JT * BL], DT, tag="t", name=f"t{it}")
                ps_tiles = []
                for p in range(3):
                    ps = ppool.tile([128, 2 * BL], F32, tag=f"ps{p}",
                                    name=f"ps{it}_{p}")
                    ps_tiles.append(ps)
                    for k in range(JT):
                        for jj in range(2):
                            mm(ps, p, jj, k, t_cur)
                    sl = slice(p * 2 * BL, (p + 1) * 2 * BL)
                    if not last:
                        nc.vector.scalar_tensor_tensor(
                            out=t_nxt[:, sl], in0=ps[:], scalar=1.0,
                            in1=negw[:, sl], op0=AT.mult, op1=AT.max)
                if not last:
                    t_cur = t_nxt
                else:
                    # y = max(negw - ps, 0) on DVE (no scalar engine at all)
                    y_sb = cpool.tile([128, JT * BL], F32)
                    for p in range(3):
                        sl = slice(p * 2 * BL, (p + 1) * 2 * BL)
                        z = spool.tile([128, 2 * BL], F32, tag="z",
                                       name=f"z{p}")
                        nc.vector.scalar_tensor_tensor(
                            out=z[:], in0=ps_tiles[p][:], scalar=-1.0,
                            in1=negw[:, sl], op0=AT.mult, op1=AT.add)
                        nc.vector.tensor_scalar_max(y_sb[:, sl], z[:], 0.0)
                    nc.sync.dma_start(out=y_out[:], in_=y_sb[:])
    return nc


def _host_prepare(d, W1, b1, W2, b2, A, b_eq, mm_np_dtype=MM_NP):
    A64 = A.astype(np.float64)
    M = np.linalg.pinv(A64 @ A64.T)
    G = A64.T @ M @ A64
    c = (b_eq.astype(np.float64) @ M) @ A64

    n2 = A.shape[1]
    NP = JT * 128
    G_pad = np.zeros((NP, NP), np.float64)
    G_pad[:n2, :n2] = G
    G_pad[CROW, :n2] = -c          # affine bias via constant-1 row of t
    G_pad[CROW, CROW] = 1.0        # keeps t[CROW] == 1 across iterations

    g_sb = np.zeros((128, JT * JT * 128), mm_np_dtype)
    for j in range(JT):
        for k in range(JT):
            pos = _gpos(j, k)
            g_sb[:, pos * 128:(pos + 1) * 128] = G_pad[
                k * 128:(k + 1) * 128, j * 128:(j + 1) * 128].astype(mm_np_dtype)

    HID = W1.shape[1]
    W2_pad = np.zeros((HID, NP), np.float64)
    W2_pad[:, :n2] = -W2.astype(np.float64)     # pre-negated
    w2_sb = (W2_pad.reshape(HT, 128, JT, 128).transpose(1, 0, 2, 3)
             .reshape(128, HT * JT * 128)).astype(mm_np_dtype)

    # W1 with b1 as a 65th contraction row
    w1_aug = np.concatenate([W1.astype(np.float64),
                             b1.astype(np.float64)[None, :]], axis=0)
    w1_host = w1_aug.astype(mm_np_dtype)        # [65, 640]

    b2_pad = np.zeros(NP, np.float32)
    b2_pad[:n2] = b2
    nb2c = (-b2_pad).reshape(JT, 128).T.astype(np.float32)   # [128, JT]
    nb2c[CROW % 128, CROW // 128] = 1.0   # negw[CROW] = 1 -> t[CROW] = 1
    nb2cb = np.repeat(nb2c[:, :, None], BL, axis=2).reshape(128, JT * BL)
    nb2cb = np.ascontiguousarray(nb2cb, np.float32)

    shared = {"g_mat": g_sb, "w2t": w2_sb, "w1": w1_host, "nb2cb": nb2cb}
    B = d.shape[0]
    bl = B // N_CORES
    in_maps = []
    for i in range(N_CORES):
        dT = d[i * bl:(i + 1) * bl, :].T.astype(mm_np_dtype)
        dT = np.concatenate([dT, np.ones((1, bl), mm_np_dtype)], axis=0)
        in_maps.append({**shared, "dt_in": np.ascontiguousarray(dT)})
    return in_maps


_nc_cache = {}


def kernel(d, W1, b1, W2, b2, A, b_eq):
    d = np.asarray(d, np.float32)
    W1 = np.asarray(W1, np.float32)
    b1 = np.asarray(b1, np.float32)
    W2 = np.asarray(W2, np.float32)
    b2 = np.asarray(b2, np.float32)
    A = np.asarray(A, np.float32)
    b_eq = np.asarray(b_eq, np.float32)

    if "nc" not in _nc_cache:
        _nc_cache["nc"] = _build(mm_dtype=MM_DTYPE)
    nc = _nc_cache["nc"]

    in_maps = _host_prepare(d, W1, b1, W2, b2, A, b_eq, mm_np_dtype=MM_NP)
    res = run_bass_kernel_spmd(nc, in_maps, list(range(N_CORES)))

    outs = []
    for r in res.results:
        y = (r["y_out"].reshape(128, JT, BL).transpose(2, 1, 0)
             .reshape(BL, JT * 128))
        outs.append(y[:, :N2])
    return np.concatenate(outs, axis=0).astype(np.float32)


# revision 6
# speedup vs baseline: 1.2872x; 1.0629x over previous
"""nn_Cvx_ShortestPathNet — TRN2 Bass kernel, 8-core pure data parallelism.

Math (collapsed Dykstra, c folded into G's padding row):
    G = A' pinv(AA') A   (orthogonal projector, 760x760), c = b' pinv(AA') A
    Key identity: c @ G = c  (since M (AA') M = M), so the affine bias -c can
    be carried by a constant-1 row in t:  Gp[CROW, :n2] = -c, Gp[CROW,CROW]=1,
    t[CROW] == 1  =>  ps = t @ Gp  computes  t@G - c  directly.
    Recurrence:  t_{k+1} = max(negw, ps_k);  y = max(negw - ps_K, 0).

On-chip layout is transposed ([768, B_local], 6x128 partition tiles) so each
matmul runs with M=128, N=32 on the PE. Per iteration: 36 matmuls in three
12-matmul j-pair groups (k-major within a group), each group followed by one
fused DVE max over [128, 64]. The group DVEs overlap the next group's
matmuls, so the PE runs near its ~27ns/matmul issue floor.

PSUM discipline: start=True clears the whole bank's has_written bits, so
only the first matmul into each psum tile carries it; later k=0 matmuls
rely on cleared has_written to overwrite.

MLP: b1 is folded into the W1 matmul via a 65th all-ones contraction row;
W2 is pre-negated on the host and b2 applied as a broadcast tensor on DVE,
so the scalar engine (and its 1.3us ACT table load) is never touched.

Batch 256 is sharded 32 rows per core; G and MLP weights replicated, all in
fp16 (fp32 accumulate in PSUM).
"""

import json
import numpy as np

import concourse.bass as bass
import concourse.mybir as mybir
import concourse.tile as tile
from concourse.bass_utils import run_bass_kernel_spmd

F32 = mybir.dt.float32
AT = mybir.AluOpType
AF = mybir.ActivationFunctionType

JT = 6          # 768/128 edge-dim tiles
BL = 32         # batch rows per core
HT = 5          # 640/128 hidden tiles
K_ITERS = 100
N_CORES = 8
N2 = 760
CROW = 760      # padding row carrying the -c bias (t[CROW] == 1)
MM_DTYPE = mybir.dt.float16
MM_NP = np.float16

# ---------------------------------------------------------------------------
# This container's walrus build rejects instructions carrying more than one
# sync-wait. Split any multi-wait instruction at the BIR-JSON level: insert
# same-engine NoOps before it, each carrying one of the extra waits (waits
# are sem-ge, so order is irrelevant).
_orig_to_json_bytes = bass.Bass.to_json_bytes
_ctr = [0]


def _split_waits_json(raw: bytes) -> bytes:
    j = json.loads(raw)
    changed = False
    for fn in j.get("functions", []):
        for bb in fn.get("blocks", []):
            out = []
            for inst in bb.get("instructions", []):
                si = inst.get("sync_info") or {}
                waits = si.get("on_wait") or []
                if len(waits) > 1:
                    changed = True
                    for w in waits[:-1]:
                        _ctr[0] += 1
                        out.append({
                            "debug": inst.get("debug", 0),
                            "engine": inst["engine"],
                            "ins": [], "outs": [],
                            "name": f"I-waitsplit-{_ctr[0]}",
                            "opcode": "NoOp",
                            "sync_info": {"on_wait": [w], "on_update": []},
                        })
                    si["on_wait"] = waits[-1:]
                out.append(inst)
            bb["instructions"] = out
    return json.dumps(j).encode() if changed else raw


def _patched_to_json_bytes(self, *a, **k):
    return _split_waits_json(_orig_to_json_bytes(self, *a, **k))


bass.Bass.to_json_bytes = _patched_to_json_bytes


def _gpos(j, k):
    """Column block index of G block (k, j) in pair-major g_mat layout."""
    return (j // 2) * 12 + k * 2 + (j % 2)


def _build(mm_dtype=MM_DTYPE, k_iters=K_ITERS):
    nc = bass.Bass("TRN2", target_bir_lowering=False, debug=False,
                   num_devices=N_CORES)
    DT = mm_dtype
    GC = JT * JT * 128          # g_mat columns, pair-major
    CH = 12 * 128               # one pair's worth of G blocks

    dt_in = nc.dram_tensor("dt_in", [65, BL], DT, kind="ExternalInput").ap()
    w1 = nc.dram_tensor("w1", [65, HT * 128], DT, kind="ExternalInput").ap()
    nb2cb = nc.dram_tensor("nb2cb", [128, JT * BL], F32,
                           kind="ExternalInput").ap()
    w2t = nc.dram_tensor("w2t", [128, HT * JT * 128], DT, kind="ExternalInput").ap()
    g_mat = nc.dram_tensor("g_mat", [128, GC], DT, kind="ExternalInput").ap()
    y_out = nc.dram_tensor("y_out", [128, JT * BL], F32, kind="ExternalOutput").ap()

    with tile.TileContext(nc) as tc:
        with (
            tc.tile_pool(name="const", bufs=1) as cpool,
            tc.tile_pool(name="state", bufs=2) as spool,
            tc.tile_pool(name="psum", bufs=2, space="PSUM") as ppool,
        ):
            dT_sb = cpool.tile([65, BL], DT)
            nc.sync.dma_start(out=dT_sb[:], in_=dt_in[:])
            w1_sb = cpool.tile([65, HT * 128], DT)
            nc.sync.dma_start(out=w1_sb[:], in_=w1[:])
            nb2cb_sb = cpool.tile([128, JT * BL], F32)
            nc.sync.dma_start(out=nb2cb_sb[:], in_=nb2cb[:])
            w2_sb = cpool.tile([128, HT * JT * 128], DT)
            nc.sync.dma_start(out=w2_sb[:], in_=w2t[:])
            G_sb = cpool.tile([128, GC], DT)
            for p in range(3):
                nc.sync.dma_start(out=G_sb[:, p * CH:(p + 1) * CH],
                                  in_=g_mat[:, p * CH:(p + 1) * CH])

            # MLP: h = leaky_relu(d@W1 + b1)  (b1 via ones-row of dT)
            h_sb = cpool.tile([128, HT * BL], DT)
            for m in range(HT):
                ph = ppool.tile([128, BL], F32, tag="mlp")
                nc.tensor.matmul(out=ph[:], lhsT=w1_sb[:, m * 128:(m + 1) * 128],
                                 rhs=dT_sb[:], start=True, stop=True)
                # leaky relu: DVE reads at most one PSUM operand, so
                # stage 0.1x in SBUF then max(x, 0.1x)
                pre = spool.tile([128, BL], F32, tag="pre", name=f"pre{m}")
                nc.vector.tensor_scalar_mul(pre[:], ph[:], 0.1)
                nc.vector.scalar_tensor_tensor(
                    out=h_sb[:, m * BL:(m + 1) * BL], in0=ph[:],
                    scalar=1.0, in1=pre[:], op0=AT.mult, op1=AT.max)

            # negw = h@(-W2) + (-b2)  into the three pair psum tiles
            negw = cpool.tile([128, JT * BL], F32)
            t_cur = spool.tile([128, JT * BL], DT, tag="t", name="t_init")
            for p in range(3):
                pw = ppool.tile([128, 2 * BL], F32, tag=f"ps{p}",
                                name=f"pw{p}")
                for jj in range(2):
                    j = 2 * p + jj
                    for k2 in range(HT):
                        nc.tensor.matmul(
                            out=pw[:, jj * BL:(jj + 1) * BL],
                            lhsT=w2_sb[:, (k2 * JT + j) * 128:
                                       (k2 * JT + j + 1) * 128],
                            rhs=h_sb[:, k2 * BL:(k2 + 1) * BL],
                            start=(jj == 0 and k2 == 0), stop=(k2 == HT - 1),
                            skip_group_check=True)
                sl = slice(p * 2 * BL, (p + 1) * 2 * BL)
                nc.vector.scalar_tensor_tensor(
                    out=negw[:, sl], in0=pw[:], scalar=1.0,
                    in1=nb2cb_sb[:, sl], op0=AT.mult, op1=AT.add)
                nc.vector.tensor_copy(out=t_cur[:, sl], in_=negw[:, sl])

            def mm(ps_tile, p, jj, k, rhs_t):
                j = 2 * p + jj
                pos = _gpos(j, k)
                nc.tensor.matmul(
                    out=ps_tile[:, jj * BL:(jj + 1) * BL],
                    lhsT=G_sb[:, pos * 128:(pos + 1) * 128],
                    rhs=rhs_t[:, k * BL:(k + 1) * BL],
                    start=(jj == 0 and k == 0), stop=(k == JT - 1),
                    skip_group_check=True)

            for it in range(k_iters):
                last = it == k_iters - 1
                t_nxt = None if last else spool.tile(
                    [128, JT * BL], DT, tag="t", name=f"t{it}")
                ps_tiles = []
                for p in range(3):
                  # virtual-clock floor: forces the scheduler to emit
                  # pair-major order (12 MMs then that pair's DVE), so each
                  # pair's max fires as early as MM#12/#24/#36 and the next
                  # iteration's matmuls never wait on a boundary DVE clump
                  with tc.tile_wait_until(0.001 * (3 * it + p + 3)):
                    ps = ppool.tile([128, 2 * BL], F32, tag=f"ps{p}",
                                    name=f"ps{it}_{p}")
                    ps_tiles.append(ps)
                    for k in range(JT):
                        for jj in range(2):
                            mm(ps, p, jj, k, t_cur)
                    sl = slice(p * 2 * BL, (p + 1) * 2 * BL)
                    if not last:
                        nc.vector.scalar_tensor_tensor(
                            out=t_nxt[:, sl], in0=ps[:], scalar=1.0,
                            in1=negw[:, sl], op0=AT.mult, op1=AT.max)
                if not last:
                    t_cur = t_nxt
                else:
                    # y = max(negw - ps, 0) on DVE (no scalar engine at all)
                    y_sb = cpool.tile([128, JT * BL], F32)
                    for p in range(3):
                        sl = slice(p * 2 * BL, (p + 1) * 2 * BL)
                        z = spool.tile([128, 2 * BL], F32, tag="z",
                                       name=f"z{p}")
                        nc.vector.scalar_tensor_tensor(
                            out=z[:], in0=ps_tiles[p][:], scalar=-1.0,
                            in1=negw[:, sl], op0=AT.mult, op1=AT.add)
                        nc.vector.tensor_scalar_max(y_sb[:, sl], z[:], 0.0)
                    nc.sync.dma_start(out=y_out[:], in_=y_sb[:])
    return nc


def _host_prepare(d, W1, b1, W2, b2, A, b_eq, mm_np_dtype=MM_NP):
    A64 = A.astype(np.float64)
    M = np.linalg.pinv(A64 @ A64.T)
    G = A64.T @ M @ A64
    c = (b_eq.astype(np.float64) @ M) @ A64

    n2 = A.shape[1]
    NP = JT * 128
    G_pad = np.zeros((NP, NP), np.float64)
    G_pad[:n2, :n2] = G
    G_pad[CROW, :n2] = -c          # affine bias via constant-1 row of t
    G_pad[CROW, CROW] = 1.0        # keeps t[CROW] == 1 across iterations

    g_sb = np.zeros((128, JT * JT * 128), mm_np_dtype)
    for j in range(JT):
        for k in range(JT):
            pos = _gpos(j, k)
            g_sb[:, pos * 128:(pos + 1) * 128] = G_pad[
                k * 128:(k + 1) * 128, j * 128:(j + 1) * 128].astype(mm_np_dtype)

    HID = W1.shape[1]
    W2_pad = np.zeros((HID, NP), np.float64)
    W2_pad[:, :n2] = -W2.astype(np.float64)     # pre-negated
    w2_sb = (W2_pad.reshape(HT, 128, JT, 128).transpose(1, 0, 2, 3)
             .reshape(128, HT * JT * 128)).astype(mm_np_dtype)

    # W1 with b1 as a 65th contraction row
    w1_aug = np.concatenate([W1.astype(np.float64),
                             b1.astype(np.float64)[None, :]], axis=0)
    w1_host = w1_aug.astype(mm_np_dtype)        # [65, 640]

    b2_pad = np.zeros(NP, np.float32)
    b2_pad[:n2] = b2
    nb2c = (-b2_pad).reshape(JT, 128).T.astype(np.float32)   # [128, JT]
    nb2c[CROW % 128, CROW // 128] = 1.0   # negw[CROW] = 1 -> t[CROW] = 1
    nb2cb = np.repeat(nb2c[:, :, None], BL, axis=2).reshape(128, JT * BL)
    nb2cb = np.ascontiguousarray(nb2cb, np.float32)

    shared = {"g_mat": g_sb, "w2t": w2_sb, "w1": w1_host, "nb2cb": nb2cb}
    B = d.shape[0]
    bl = B // N_CORES
    in_maps = []
    for i in range(N_CORES):
        dT = d[i * bl:(i + 1) * bl, :].T.astype(mm_np_dtype)
        dT = np.concatenate([dT, np.ones((1, bl), mm_np_dtype)], axis=0)
        in_maps.append({**shared, "dt_in": np.ascontiguousarray(dT)})
    return in_maps


_nc_cache = {}


def kernel(d, W1, b1, W2, b2, A, b_eq):
    d = np.asarray(d, np.float32)
    W1 = np.asarray(W1, np.float32)
    b1 = np.asarray(b1, np.float32)
    W2 = np.asarray(W2, np.float32)
    b2 = np.asarray(b2, np.float32)
    A = np.asarray(A, np.float32)
    b_eq = np.asarray(b_eq, np.float32)

    if "nc" not in _nc_cache:
        _nc_cache["nc"] = _build(mm_dtype=MM_DTYPE)
    nc = _nc_cache["nc"]

    in_maps = _host_prepare(d, W1, b1, W2, b2, A, b_eq, mm_np_dtype=MM_NP)
    res = run_bass_kernel_spmd(nc, in_maps, list(range(N_CORES)))

    outs = []
    for r in res.results:
        y = (r["y_out"].reshape(128, JT, BL).transpose(2, 1, 0)
             .reshape(BL, JT * 128))
        outs.append(y[:, :N2])
    return np.concatenate(outs, axis=0).astype(np.float32)
